# revision 12
# baseline (speedup 1.0000x reference)
"""Trainium2 Bass kernel for CustomCrossEntropyLoss (data-parallel over 8 NeuronCores).

Math (matches the reference):
    mask  = (target != 4)
    lse   = logsumexp(pred, axis=C)        # no max-subtraction: pred ~ N(0,1)
    p_t   = pred[target]   (one-hot gather)
    w     = 1.0 where ME == 0 else 0.5
    loss  = sum(w * mask * (lse - p_t)) / sum(mask)

Per-core (2 batches): each batch is a group processed in a C-major stripe
layout: tile [128, 4, 2048] holds class c of pixel-chunk j at partition
32*c + j, so exp() is one ACT pass, the one-hot gather is ONE fused
compare-multiply (scalar_tensor_tensor with a per-partition class id), and
both the class-sum (Se) and the gathered exp (Sx = exp(p_t)) reduce over
classes on the idle TensorEngine via a tiny [128]->[32] 0/1 matmul whose
output stitches back into a B-major [128, 2048] PSUM image.  ln(Se) is the
logsumexp, ln(Sx+eps) is p_t (eps keeps ignored pixels finite; their weight
num = (ME-2)*mask is 0).  diff = lse - p_t runs on GpSimd; a single DVE STT
accumulator forms sum(num*diff) and the mask pass accumulates the count.
Host sums 8 x [128, 4] stats and divides.
"""
import sys

sys.path.insert(0, "/opt/trn_rl_repo")

import numpy as np
from contextlib import ExitStack

import concourse.bacc as bacc
import concourse.tile as tile
from concourse import mybir
from concourse.bass_utils import run_bass_kernel_spmd

N_CORES = 8
B, C, H, W = 16, 4, 512, 512
HW = H * W                      # 262144 pixels per batch
BPC = B // N_CORES              # 2 batches per core
NG = BPC                        # 2 groups per core (one batch each)
F = HW // 128                   # 2048 free-dim columns per group
A = mybir.AluOpType
DT = mybir.dt
ACTF = mybir.ActivationFunctionType

_nc_cache = None


def _build():
    nc = bacc.Bacc()
    pred = nc.dram_tensor("pred", [BPC, C, HW], DT.float32, kind="ExternalInput")
    targ = nc.dram_tensor("targ", [BPC, HW], DT.int32, kind="ExternalInput")
    me = nc.dram_tensor("me", [BPC, HW], DT.int32, kind="ExternalInput")
    stats = nc.dram_tensor("stats", [128, 2 * NG], DT.float32, kind="ExternalOutput")

    with tile.TileContext(nc) as tc:
        with ExitStack() as ctx:
            big = ctx.enter_context(tc.tile_pool(name="big", bufs=2))
            mid = ctx.enter_context(tc.tile_pool(name="mid", bufs=2))
            scratch = ctx.enter_context(tc.tile_pool(name="scratch", bufs=1))
            psum = ctx.enter_context(tc.tile_pool(name="psum", bufs=1, space="PSUM"))
            singles = ctx.enter_context(tc.tile_pool(name="singles", bufs=1))

            # stats cols: [0:NG) = sum(num*diff), [NG:2NG) = mask count
            stats_t = singles.tile([128, 2 * NG], DT.float32)
            # keeps Ln finite on ignored pixels (their one-hot row sums to 0)
            eps_t = singles.tile([128, 1], DT.float32)
            nc.vector.memset(eps_t, 1e-30)

            # cls_vec[p] = p // 32 (class id of each partition stripe)
            cls_vec = singles.tile([128, 1], DT.float32)
            for c in range(C):
                nc.vector.memset(cls_vec[32 * c : 32 * (c + 1), :], float(c))

            # sel[p, j] = (p % 32 == j): [128]->[32] class-sum matmul weights
            selm = singles.tile([128, 32], DT.bfloat16)
            tmp_i = singles.tile([128, 32], DT.int32)
            nc.gpsimd.iota(tmp_i, pattern=[[-1, 32]], base=0, channel_multiplier=1)
            sel_parts = []
            for k in range(4):
                sk = singles.tile([128, 32], DT.bfloat16, tag=f"selk{k}")
                nc.vector.tensor_scalar(
                    out=sk, in0=tmp_i, scalar1=32 * k, scalar2=None, op0=A.is_equal
                )
                sel_parts.append(sk)
            nc.vector.tensor_tensor(
                out=sel_parts[0], in0=sel_parts[0], in1=sel_parts[1], op=A.add
            )
            nc.vector.tensor_tensor(
                out=sel_parts[2], in0=sel_parts[2], in1=sel_parts[3], op=A.add
            )
            nc.vector.tensor_tensor(
                out=selm, in0=sel_parts[0], in1=sel_parts[2], op=A.add
            )

            for h in range(NG):
                # pred in C-major stripes: row 32c+j, free (g, n) covers pixel
                # 32768*g + 2048*j + n of batch h, class c (8 KB contiguous runs)
                p_t = big.tile([128, C, F], DT.float32, tag="p")
                for c in range(C):
                    for hf in range(2):
                        px0 = hf * (HW // 2)
                        nc.sync.dma_start(
                            out=p_t[32 * c : 32 * (c + 1), 2 * hf : 2 * hf + 2, :],
                            in_=pred[h, c, px0 : px0 + HW // 2].rearrange(
                                "(g j n) -> j g n", j=32, n=F
                            ),
                        )
                t_i = big.tile([128, F], DT.int32, tag="t")
                nc.scalar.dma_start(
                    out=t_i, in_=targ[h, :].rearrange("(q n) -> q n", n=F)
                )
                m_i = big.tile([128, F], DT.int32, tag="m")
                nc.scalar.dma_start(
                    out=m_i, in_=me[h, :].rearrange("(q n) -> q n", n=F)
                )

                t_bf = scratch.tile([128, F], DT.bfloat16, tag="tbf")
                nc.scalar.activation(out=t_bf, in_=t_i, func=ACTF.Copy)
                mask = scratch.tile([128, F], DT.bfloat16, tag="mask")
                nc.vector.tensor_scalar(
                    out=mask, in0=t_bf, scalar1=4.0, scalar2=None,
                    op0=A.is_lt, op1=A.add,
                    accum_out=stats_t[:, NG + h : NG + h + 1],
                )
                num = scratch.tile([128, F], DT.bfloat16, tag="num")
                nc.vector.scalar_tensor_tensor(
                    out=num, in0=m_i, scalar=2.0, in1=mask,
                    op0=A.subtract, op1=A.mult,
                )

                # replicate target stripes to the 4 class bands (on-chip, SWDGE)
                trep = scratch.tile([128, C, F], DT.bfloat16, tag="trep")
                for g in range(4):
                    for c in range(C):
                        nc.gpsimd.dma_start(
                            out=trep[32 * c : 32 * (c + 1), g, :],
                            in_=t_bf[32 * g : 32 * (g + 1), :],
                        )

                # E = exp(pred) bf16, one half (all classes) at a time so DVE
                # work starts after 2 MB instead of 4 MB of pred
                e_t = mid.tile([128, C, F], DT.bfloat16, tag="e")
                x_t = scratch.tile([128, C, F], DT.bfloat16, tag="x")
                for hf in range(2):
                    sl2 = slice(2 * hf, 2 * hf + 2)
                    nc.scalar.activation(
                        out=e_t[:, sl2, :], in_=p_t[:, sl2, :], func=ACTF.Exp
                    )
                    # one-hot via 4x-mode tensor_scalar, then in-place 2x TT mult
                    nc.vector.tensor_scalar(
                        out=x_t[:, sl2, :], in0=trep[:, sl2, :],
                        scalar1=cls_vec, scalar2=None, op0=A.is_equal,
                    )
                    nc.vector.tensor_tensor(
                        out=x_t[:, sl2, :], in0=x_t[:, sl2, :],
                        in1=e_t[:, sl2, :], op=A.mult,
                    )

                # class sums on the TensorEngine -> B-major [128, 2048] PSUM
                s_ps = psum.tile([128, F], DT.float32, tag="s")
                pt_ps = psum.tile([128, F], DT.float32, tag="pt")
                for g in range(4):
                    for k in range(F // 512):
                        sl = slice(512 * k, 512 * (k + 1))
                        nc.tensor.matmul(
                            out=s_ps[32 * g : 32 * g + 32, sl], lhsT=selm,
                            rhs=e_t[:, g, sl], start=True, stop=True,
                            tile_position=(0, 32 * g),
                        )
                        nc.tensor.matmul(
                            out=pt_ps[32 * g : 32 * g + 32, sl], lhsT=selm,
                            rhs=x_t[:, g, sl], start=True, stop=True,
                            tile_position=(0, 32 * g),
                        )

                lse = scratch.tile([128, F], DT.bfloat16, tag="lse")
                nc.scalar.activation(out=lse, in_=s_ps, func=ACTF.Ln)
                lnpt = scratch.tile([128, F], DT.bfloat16, tag="lnpt")
                nc.scalar.activation(out=lnpt, in_=pt_ps, func=ACTF.Ln, bias=eps_t)

                # diff = lse - p_t on GpSimd (frees the DVE), in place into lse
                nc.gpsimd.tensor_tensor(
                    out=lse, in0=lse, in1=lnpt, op=A.subtract
                )
                nc.vector.scalar_tensor_tensor(
                    out=x_t[:, 0, :], in0=num, scalar=0.0, in1=lse,
                    op0=A.bypass, op1=A.mult,
                    accum_out=stats_t[:, h : h + 1],
                )

            nc.sync.dma_start(out=stats[:, :], in_=stats_t)
    nc.finalize()
    return nc


def _get_nc():
    global _nc_cache
    if _nc_cache is None:
        _nc_cache = _build()
    return _nc_cache


def _install_ntff_hook():
    """Register the axon NTFF profiling hook (missing antenv.axon_hooks glue)."""
    import types
    import ctypes
    import contextlib

    try:
        from antenv.axon_hooks import get_axon_ntff_profile_hook  # noqa: F401

        return
    except ImportError:
        pass

    so_path = "/opt/axon/libaxon_pjrt.so"
    try:
        lib = ctypes.CDLL(so_path)
    except OSError:
        return
    if not hasattr(lib, "axon_start_nrt_profile"):
        return
    lib.axon_start_nrt_profile.argtypes = [
        ctypes.POINTER(ctypes.c_int64),
        ctypes.c_size_t,
    ]
    lib.axon_start_nrt_profile.restype = ctypes.c_int64
    lib.axon_stop_nrt_profile.argtypes = [ctypes.c_char_p]
    lib.axon_stop_nrt_profile.restype = ctypes.c_int64

    @contextlib.contextmanager
    def _hook(output_dir, device_ids):
        import jax

        jax.devices()
        if device_ids:
            ids = (ctypes.c_int64 * len(device_ids))(*device_ids)
            rc = lib.axon_start_nrt_profile(ids, len(device_ids))
        else:
            rc = lib.axon_start_nrt_profile(None, 0)
        if rc != 0:
            raise RuntimeError(f"axon_start_nrt_profile rc={rc}")
        try:
            yield
        finally:
            n = lib.axon_stop_nrt_profile(str(output_dir).encode())
            print(f"ntff profile: {n} file(s) -> {output_dir}")

    mod = types.ModuleType("antenv.axon_hooks")
    mod.get_axon_ntff_profile_hook = lambda: _hook
    mod.set_axon_ntff_profile_hook = lambda h: None
    sys.modules["antenv.axon_hooks"] = mod

    from concourse import bass_utils as _bu

    _bu.upload_artifacts = lambda tmpdir: tmpdir


def _run(pred, target, ME, trace=False, tmpdir=None):
    pred = np.ascontiguousarray(pred, dtype=np.float32).reshape(B, C, HW)
    target = np.ascontiguousarray(target, dtype=np.int32).reshape(B, HW)
    ME = np.ascontiguousarray(ME, dtype=np.int32).reshape(B, HW)

    in_maps = []
    for i in range(N_CORES):
        sl = slice(i * BPC, (i + 1) * BPC)
        in_maps.append(
            {
                "pred": np.ascontiguousarray(pred[sl]),
                "targ": np.ascontiguousarray(target[sl]),
                "me": np.ascontiguousarray(ME[sl]),
            }
        )

    nc = _get_nc()
    if trace:
        _install_ntff_hook()
    res = run_bass_kernel_spmd(
        nc, in_maps, core_ids=list(range(N_CORES)), trace=trace, tmpdir=tmpdir
    )

    acc_d = cnt = 0.0
    for i in range(N_CORES):
        st = res.results[i]["stats"].astype(np.float64)
        acc_d += st[:, 0:NG].sum()
        cnt += st[:, NG : 2 * NG].sum()

    # num = (ME-2)*mask = -2*w*mask  =>  sum(w*mask*diff) = -acc_d/2
    loss = -acc_d / (2.0 * cnt)
    return np.float32(loss), res.exec_time_ns


def kernel(pred, target, ME):
    loss, _ = _run(pred, target, ME, trace=False)
    return loss
